# revision 26
# baseline (speedup 1.0000x reference)
"""Trainium2 Bass kernel for nn_Attn (Bahdanau-style attention scores).

Computation (per batch b of B=128):
    energy = tanh(enc[b] @ We.T + (hidden @ Wh.T)[b] + bias)   # (L, H)
    scores = energy @ v                                        # (L,)
    out[b] = softmax(scores)                                   # (1, L)

Sharding: batch data-parallel over 8 NeuronCores (16 batches/core);
weights replicated. Per core the dominant matmul is computed in the
[h, l] orientation so the PE tensor engine contracts over d (=576):

    part_e[h, l] = sum_d WeT[d, h] * encT[d, l]      (lhsT=WeT, rhs=encT)

which lets the (hidden@Wh.T + bias) term fuse into the tanh as a
per-partition activation bias, and the v-contraction run as a second
PE matmul (lhsT = v column, contracting over h on partitions).
Matmuls run as float32r (full fp32 data, reduced-precision multiply,
1 col/cycle on TRN2 vs 4 for exact fp32).

The contraction dim is zero-padded host-side from 576 to 640 so every
k-tile is a full 128 partitions: K=64 matmuls (and their successors)
measure ~2x slower on HW than K=128 ones, costing far more than the 11%
extra DMA.

Host side: encoder_outputs (L, B, D) is transposed once to (B, D, L) so
each per-batch d-major tile DMA is contiguous.

Scores are assembled batch-major ([16, L] via tiny SBUF->SBUF row DMAs)
so softmax runs once over all local batches at the end instead of as 16
serial per-batch chains on the ACT/DVE engines.
"""

import numpy as np

import concourse.bacc as bacc
import concourse.bass as bass
import concourse.mybir as mybir
import concourse.tile as tile
from concourse import bass_utils
from concourse.mybir import ActivationFunctionType as AF
from concourse.mybir import AluOpType, AxisListType

N_CORES = 8
B, L, H = 128, 1024, 512
ONEHOT = 64
DE = H + ONEHOT          # 576, true contraction dim of the big matmul
DP = 640                 # padded contraction dim (5 full 128-tiles)
BL = B // N_CORES        # 16 batches per core
F32 = mybir.dt.float32
F32R = mybir.dt.float32r

NKT = DP // 128                          # 5 d-tiles, all full
NHT = H // 128                           # 4 h-tiles
NLH = L // 512                           # 2 l-halves (N=512 per matmul)


BF16 = mybir.dt.bfloat16


def build(reps: int = 1, dt1=F32R, dt2=F32R, dve2: bool = True):
    """Build + trace the per-core Bass program. Returns the compiled nc.

    dt1: dtype of the stage-1 matmul operands (enc tiles + We tiles).
    dt2: dtype of the stage-2 operands (energy tiles + v columns).
    dve2: compute z[p,l] = sum_ht v_ht[p]*en_ht[p,l] on the VectorE
        (per-partition scalar multiply-accumulate), so stage-2 on the PE
        collapses from 4 matmuls to a single K=128 ones-matmul per
        (batch, l-half). Saves ~23us of PE time for ~45us of idle DVE.
    """
    nc = bacc.Bacc(
        "TRN2", target_bir_lowering=False, debug=False, num_devices=N_CORES
    )
    enc = nc.dram_tensor("enc", [BL, DP, L], dt1, kind="ExternalInput").ap()
    hid = nc.dram_tensor("hid", [H, BL], F32, kind="ExternalInput").ap()
    wet = nc.dram_tensor("wet", [DP, H], dt1, kind="ExternalInput").ap()
    wht = nc.dram_tensor("wht", [H, H], F32, kind="ExternalInput").ap()
    bcol = nc.dram_tensor("bcol", [128, NHT], F32, kind="ExternalInput").ap()
    vcol = nc.dram_tensor("vcol", [128, NHT], F32 if dve2 else dt2, kind="ExternalInput").ap()
    ones = None
    vcolr = None
    if dve2:
        ones = nc.dram_tensor("ones", [128, 1], dt2, kind="ExternalInput").ap()
        vcolr = nc.dram_tensor("vcolr", [128, NHT], dt2, kind="ExternalInput").ap()
    out = nc.dram_tensor("out", [BL, L], F32, kind="ExternalOutput").ap()

    with tile.TileContext(nc) as tc:
        with (
            tc.tile_pool(name="const", bufs=1) as cpool,
            tc.tile_pool(name="encp", bufs=4) as epool,
            tc.tile_pool(name="energy", bufs=8) as gpool,
            tc.tile_pool(name="cb", bufs=2) as cbpool,
            tc.tile_pool(name="soft", bufs=1) as spool,
            tc.tile_pool(name="stage", bufs=4) as stpool,
            tc.tile_pool(name="ps1", bufs=6, space="PSUM") as ps1,
            tc.tile_pool(name="ps2", bufs=2, space="PSUM") as ps2,
            tc.tile_pool(name="ps3", bufs=2, space="PSUM") as ps3,
        ):
            # ---- replicated constants (gpsimd queue: don't serialize
            # behind the big enc prefetches on the sync queue) ----
            wet_sb = []
            for kt in range(NKT):
                t = cpool.tile([128, H], dt1, tag=f"wet{kt}", name=f"wet{kt}")
                nc.sync.dma_start(t[:], wet[kt * 128 : (kt + 1) * 128, :])
                wet_sb.append(t)
            wht_sb = []
            for kt in range(4):
                t = cpool.tile([128, H], F32, tag=f"wht{kt}", name=f"wht{kt}")
                nc.sync.dma_start(t[:], wht[kt * 128 : (kt + 1) * 128, :])
                wht_sb.append(t)
            hid_sb = []
            for kt in range(4):
                t = cpool.tile([128, BL], F32, tag=f"hid{kt}", name=f"hid{kt}")
                nc.sync.dma_start(t[:], hid[kt * 128 : (kt + 1) * 128, :])
                hid_sb.append(t)
            bcol_sb = cpool.tile([128, NHT], F32, tag="bcol", name="bcol_sb")
            nc.sync.dma_start(bcol_sb[:], bcol[:, :])
            vcol_sb = cpool.tile([128, NHT], F32 if dve2 else dt2, tag="vcol", name="vcol_sb")
            nc.sync.dma_start(vcol_sb[:], vcol[:, :])
            ones128 = None
            vcolr_sb = None
            if dve2:
                ones128 = cpool.tile([128, 1], dt2, tag="ones128", name="ones128")
                nc.sync.dma_start(ones128[:], ones[:, :])
                vcolr_sb = cpool.tile([128, NHT], dt2, tag="vcolr", name="vcolr_sb")
                nc.sync.dma_start(vcolr_sb[:], vcolr[:, :])

            for _rep in range(reps):
                # ---- PE warmup: high-duty junk matmuls as soon as wet lands,
                # so the HAM clock-gate reaches 8/8 before real work. The
                # N=16 c matmuls below have ~3% array duty and never warm it.
                warm = ps1.tile([128, 512], F32, tag="ps1", name="warm")
                for w in range(16):
                    nc.tensor.matmul(
                        warm[:],
                        lhsT=wet_sb[0][:, 0:128],
                        rhs=wet_sb[0][:],
                        start=(w == 0),
                        stop=(w == 15),
                    )

                # ---- c[h, b] = (hidden @ Wh.T).T + bias, per-partition h ----
                cb_sb = []
                for ht in range(4):
                    pc = ps1.tile([128, 512], F32, tag="ps1", name=f"pc{ht}")
                    for kt in range(4):
                        nc.tensor.matmul(
                            pc[:, :BL],
                            lhsT=wht_sb[kt][:, ht * 128 : (ht + 1) * 128],
                            rhs=hid_sb[kt][:],
                            start=(kt == 0),
                            stop=(kt == 3),
                        )
                    cbt = cbpool.tile([128, BL], F32, tag=f"cb{ht}", name=f"cb{ht}")
                    nc.vector.tensor_scalar_add(
                        cbt[:], pc[:, :BL], bcol_sb[:, ht : ht + 1]
                    )
                    cb_sb.append(cbt)

                scores_sb = spool.tile([BL, L], F32, tag="scores", name="scores_sb")

                # ---- main loop over local batches ----
                for b in range(BL):
                    et = []
                    for kt in range(NKT):
                        t = epool.tile([128, L], dt1, tag=f"enc{kt}", name=f"enc{kt}_{b}")
                        nc.sync.dma_start(
                            t[:], enc[b, kt * 128 : (kt + 1) * 128, :]
                        )
                        et.append(t)

                    for lh in range(NLH):
                        ens = []
                        for ht in range(4):
                            pe_t = ps1.tile(
                                [128, 512], F32, tag="ps1", name=f"pe{b}_{lh}_{ht}"
                            )
                            for kt in range(NKT):
                                nc.tensor.matmul(
                                    pe_t[:],
                                    lhsT=wet_sb[kt][:, ht * 128 : (ht + 1) * 128],
                                    rhs=et[kt][:, lh * 512 : (lh + 1) * 512],
                                    start=(kt == 0),
                                    stop=(kt == NKT - 1),
                                )
                            en_t = gpool.tile(
                                [128, 512], dt2, tag="en", name=f"en{b}_{lh}_{ht}"
                            )
                            nc.scalar.activation(
                                en_t[:], pe_t[:], AF.Tanh,
                                bias=cb_sb[ht][:, b : b + 1],
                            )
                            ens.append(en_t)
                        if dve2 and b < BL - 2:
                            # z[p, l] = sum_ht v_ht[p] * en_ht[p, l]  (DVE).
                            # Intermediates accumulate in plain f32; only the
                            # final tile is written as dt2 for the ones-matmul.
                            z = None
                            for ht in range(4):
                                zn = stpool.tile(
                                    [128, 512], dt2 if ht == 3 else F32, tag="z",
                                    name=f"z{b}_{lh}_{ht}", bufs=8,
                                )
                                if z is None:
                                    nc.vector.tensor_scalar_mul(
                                        zn[:], ens[ht][:], vcol_sb[:, ht : ht + 1]
                                    )
                                else:
                                    nc.vector.scalar_tensor_tensor(
                                        zn[:], ens[ht][:],
                                        vcol_sb[:, ht : ht + 1], z[:],
                                        AluOpType.mult, AluOpType.add,
                                    )
                                z = zn
                            # scores[l] = sum_p z[p, l]: one K=128 ones-matmul
                            ps_s = ps3.tile(
                                [1, 512], F32, tag="pss", name=f"ps_s{b}_{lh}"
                            )
                            nc.tensor.matmul(
                                ps_s[:], lhsT=ones128[:], rhs=z[:],
                                start=True, stop=True,
                            )
                        else:
                            vc = vcolr_sb if dve2 else vcol_sb
                            pspool = ps3 if dve2 else ps2
                            ps_s = pspool.tile(
                                [1, 512], F32, tag="pss", name=f"ps_s{b}_{lh}"
                            )
                            for ht in range(4):
                                nc.tensor.matmul(
                                    ps_s[:],
                                    lhsT=vc[:, ht : ht + 1],
                                    rhs=ens[ht][:],
                                    start=(ht == 0),
                                    stop=(ht == 3),
                                )
                        # stage psum scores out and park them batch-major
                        st = stpool.tile([1, 512], F32, tag="st", name=f"st{b}_{lh}")
                        nc.vector.tensor_copy(st[:], ps_s[:])
                        nc.sync.dma_start(
                            scores_sb[b : b + 1, lh * 512 : (lh + 1) * 512], st[:]
                        )

                # ---- one softmax over all local batches ----
                mx = spool.tile([BL, 1], F32, tag="mx", name="mx")
                nc.vector.tensor_reduce(
                    mx[:], scores_sb[:], axis=AxisListType.X, op=AluOpType.max,
                    negate=True,
                )
                ex = spool.tile([BL, L], F32, tag="ex", name="ex")
                sm = spool.tile([BL, 1], F32, tag="sm", name="sm")
                nc.scalar.activation(
                    ex[:], scores_sb[:], AF.Exp, bias=mx[:, 0:1],
                    accum_out=sm[:],
                )
                rc = spool.tile([BL, 1], F32, tag="rc", name="rc")
                nc.vector.reciprocal(rc[:], sm[:])
                oo = spool.tile([BL, L], F32, tag="oo", name="oo")
                nc.vector.tensor_scalar_mul(oo[:], ex[:], rc[:, 0:1])
                nc.sync.dma_start(out[:, :], oo[:])

    nc.compile()
    return nc


_cached_nc = None


def _prep_in_maps(hidden, encoder_outputs, W, b, v, np1=np.float32, np2=np.float32):
    hidden = np.ascontiguousarray(hidden, dtype=np.float32)
    W = np.ascontiguousarray(W, dtype=np.float32)
    b = np.ascontiguousarray(b, dtype=np.float32)
    v = np.ascontiguousarray(v, dtype=np.float32)
    # (L, B, D) -> (B, D, L), zero-padded to DP on the d axis
    e = np.asarray(encoder_outputs, dtype=np.float32)
    encT = np.zeros((B, DP, L), dtype=np1)
    encT[:, :DE, :] = e.transpose(1, 2, 0).astype(np1)
    wet = np.zeros((DP, H), dtype=np1)
    wet[:DE] = W[:, H:].T.astype(np1)                   # We.T (padded)
    wht = np.ascontiguousarray(W[:, :H].T)              # (512, 512)
    bcol = np.ascontiguousarray(b.reshape(NHT, 128).T)  # (128, 4)
    vcol = np.ascontiguousarray(v.reshape(NHT, 128).T).astype(np2)  # (128, 4)
    ones = np.ones((128, 1), dtype=np1)
    in_maps = []
    for c in range(N_CORES):
        sl = slice(c * BL, (c + 1) * BL)
        in_maps.append(
            {
                "enc": encT[sl],
                "hid": np.ascontiguousarray(hidden[sl].T),
                "wet": wet,
                "wht": wht,
                "bcol": bcol,
                "vcol": vcol,
                "ones": ones,
                "vcolr": vcol.astype(np1),
            }
        )
    return in_maps


def kernel(hidden, encoder_outputs, W, b, v):
    global _cached_nc
    if _cached_nc is None:
        _cached_nc = build(reps=1)
    in_maps = _prep_in_maps(hidden, encoder_outputs, W, b, v)
    res = bass_utils.run_bass_kernel_spmd(
        _cached_nc, in_maps, core_ids=list(range(N_CORES))
    )
    outs = np.concatenate([res.results[c]["out"] for c in range(N_CORES)], axis=0)
    return outs[:, None, :].astype(np.float32)
